# revision 11
# baseline (speedup 1.0000x reference)
"""Multihead causal attention block on 8 Trainium2 NeuronCores.

Sharding: tensor-parallel over heads (2 heads/core). Each core computes
qkv + attention for its heads over all tokens; two AllToAlls (one per
batch element, pipelined against attention compute) redistribute
attention outputs so each core holds all 1024 feature dims for two
256-token half-slices, where it runs the output projection locally.

Fixed problem shape: B=2, T=2048, C=1024, H=16, HS=64.
"""

import sys

sys.path.insert(0, "/opt/trn_rl_repo")

import numpy as np
import ml_dtypes

import concourse.bass as bass
import concourse.tile as tile
from concourse import bacc, mybir
from concourse import bass_utils

B, T, C = 2, 2048, 1024
H, HS = 16, 64
G = B * T              # 4096 global tokens (b-major)
NCORES = 8
NKT = C // 128         # 8 contraction tiles
HTS = T // NCORES      # 256-token half-slice per core per batch

dt = mybir.dt
BF = dt.bfloat16
F32 = dt.float32
EXP = mybir.ActivationFunctionType.Exp

_CACHED = None


def _build():
    nc = bacc.Bacc("TRN2", target_bir_lowering=False, debug=False,
                   num_devices=NCORES)

    xT_d = nc.dram_tensor("xT", [C, G], BF, kind="ExternalInput")
    wqkv_d = nc.dram_tensor("wqkv", [C, 384], BF, kind="ExternalInput")
    wpT_d = nc.dram_tensor("wpT", [C, C], BF, kind="ExternalInput")
    bias_d = nc.dram_tensor("bias", [128, C], F32, kind="ExternalInput")
    mask_d = nc.dram_tensor("mask", [128, 896], BF, kind="ExternalInput")
    y_d = nc.dram_tensor("y", [2 * HTS, C], F32, kind="ExternalOutput")

    a2a_shapes = [HTS, 128, 128]
    a2a_in = [nc.dram_tensor(f"a2a_in{w}", [NCORES * 128, a2a_shapes[w]], BF)
              for w in range(3)]
    a2a_out = [nc.dram_tensor(f"a2a_out{w}", [NCORES * 128, a2a_shapes[w]], BF)
               for w in range(3)]

    with tile.TileContext(nc) as tc:
        with tc.tile_pool(name="cst", bufs=1) as cst, \
             tc.tile_pool(name="pt", bufs=3) as ptp, \
             tc.tile_pool(name="sm", bufs=4) as smp, \
             tc.tile_pool(name="yp", bufs=3) as yp, \
             tc.tile_pool(name="psS", bufs=2, space="PSUM") as psS, \
             tc.tile_pool(name="psPV", bufs=2, space="PSUM") as psPV, \
             tc.tile_pool(name="psQ", bufs=2, space="PSUM") as psQ:

            # ---- constant loads -------------------------------------
            x_sb = cst.tile([128, NKT * G], BF)        # x^T c-tiles
            w_sb = cst.tile([128, NKT * 384], BF)      # per-head qkv weights
            mask_sb = cst.tile([128, 896], BF)
            bias_sb = cst.tile([128, C], F32)
            wp_sb = cst.tile([128, NKT * C], BF)       # w_proj^T c-tiles
            qT = cst.tile([128, G], BF)                # q^T (2 heads stacked)
            kT = cst.tile([128, G], BF)
            v_sb = cst.tile([128, 32 * 130], BF)       # v natural + ones col
            staged = cst.tile([128, G], BF)            # normalized out^T
            ones_sb = cst.tile([1, 64], BF)
            po0 = cst.tile([128, NCORES * 256], BF)    # b0 tokens 256j
            po1a = cst.tile([128, NCORES * 128], BF)   # b1 tokens 128j
            po1b = cst.tile([128, NCORES * 128], BF)   # b1 tokens 1024+128j

            nc.sync.dma_start(
                w_sb[:].rearrange("p (k n) -> p k n", k=NKT),
                wqkv_d[:].rearrange("(k p) n -> p k n", p=128))
            # x in priority waves: tokens 0-511 first (unblocks the
            # pipeline), then rest of b0, then b1; spread issue across
            # the three DMA-capable engines
            dma_engines = [nc.sync, nc.gpsimd, nc.scalar]
            di = 0
            for w0, w1 in ((0, 512), (512, 1024), (1024, T), (T, 3 * 1024), (3 * 1024, G)):
                for kk in range(NKT):
                    eng = dma_engines[di % len(dma_engines)]
                    di += 1
                    eng.dma_start(
                        x_sb[:, kk * G + w0: kk * G + w1],
                        xT_d[128 * kk: 128 * (kk + 1), w0: w1])
            nc.sync.dma_start(mask_sb[:], mask_d[:])
            nc.sync.dma_start(bias_sb[:], bias_d[:])
            for kk in range(NKT):
                nc.sync.dma_start(wp_sb[:, kk * C:(kk + 1) * C],
                                  wpT_d[128 * kk:128 * (kk + 1), :])

            nc.vector.memset(ones_sb[:], 1.0)
            # ones column of v_aug: offsets 64 + 65*m
            v_ones = v_sb[:].rearrange("p (m o) -> p m o", o=65)[:, :, 64:65]
            nc.vector.memset(v_ones, 1.0)

            # ---- qkv projections ------------------------------------
            def qkv_block(b, tb0=0, tb1=4):
                for tb in range(tb0, tb1):
                    gt = b * T + tb * 512
                    for part in range(2):      # 0=q pair, 1=k pair
                        ps = psQ.tile([128, 512], F32, tag="q", name="psqk")
                        for kk in range(NKT):
                            nc.tensor.matmul(
                                ps[:],
                                w_sb[:, kk * 384 + part * 128:
                                     kk * 384 + part * 128 + 128],
                                x_sb[:, kk * G + gt: kk * G + gt + 512],
                                start=(kk == 0), stop=(kk == NKT - 1))
                        dst = qT if part == 0 else kT
                        nc.vector.tensor_copy(dst[:, gt:gt + 512], ps[:])
                    for ts in range(4):        # v in natural layout
                        g0 = gt + 128 * ts
                        jb = g0 // 128
                        ps = psQ.tile([128, 128], F32, tag="q", name="psv")
                        for kk in range(NKT):
                            nc.tensor.matmul(
                                ps[:],
                                x_sb[:, kk * G + g0: kk * G + g0 + 128],
                                w_sb[:, kk * 384 + 256: kk * 384 + 384],
                                start=(kk == 0), stop=(kk == NKT - 1))
                        dst = v_sb[:, 130 * jb: 130 * jb + 130] \
                            .rearrange("p (h o) -> p h o", o=65)[:, :, 0:64]
                        nc.vector.tensor_copy(
                            dst, ps[:].rearrange("p (h d) -> p h d", d=64))

            # ---- attention for one (b, I) i-block of 512 ------------
            def attn_block(b, I):
                icol = (b * 4 + I) * 512
                pv = [psPV.tile([65, 512], F32, tag="pv", name=f"pv{b}{I}{hh}")
                      for hh in range(2)]
                # off-diagonal j-blocks, chunks of 2 (no mask needed)
                for cc in range(2 * I):
                    pss = [psS.tile([128, 1024], F32, tag="s",
                                    name=f"pss{hh}") for hh in range(2)]
                    for u in range(2):
                        for h in range(2):
                            jb = b * 16 + 2 * cc + u
                            nc.tensor.matmul(
                                pss[h][:, u * 512:(u + 1) * 512],
                                kT[h * 64:(h + 1) * 64,
                                   jb * 128: jb * 128 + 128],
                                qT[h * 64:(h + 1) * 64, icol: icol + 512],
                                start=True, stop=True,
                                tile_position=(h * 64, 0))
                    for h in range(2):
                        pt = ptp.tile([128, 1024], BF, tag="pt", name="pt")
                        nc.scalar.activation(pt[:], pss[h][:], EXP)
                        for u in range(2):
                            jb = b * 16 + 2 * cc + u
                            nc.tensor.matmul(
                                pv[h][:],
                                v_sb[:, 130 * jb + 65 * h:
                                     130 * jb + 65 * h + 65],
                                pt[:, u * 512:(u + 1) * 512],
                                start=(cc == 0 and u == 0), stop=False)
                # diagonal j-blocks: shrink to valid columns, batch rr
                # pairs into one psum tile / one exp, triangle mask
                for rp in range(2):            # rr pair: (0,1) or (2,3)
                    rrs = (2 * rp, 2 * rp + 1)
                    ws = [512 - 128 * rr for rr in rrs]
                    pss = [psS.tile([128, 1024], F32, tag="s",
                                    name=f"psd{hh}") for hh in range(2)]
                    for ui, rr in enumerate(rrs):
                        off = 128 * rr
                        c0 = 0 if ui == 0 else ws[0]
                        for h in range(2):
                            jb = b * 16 + 4 * I + rr
                            nc.tensor.matmul(
                                pss[h][:, c0: c0 + ws[ui]],
                                kT[h * 64:(h + 1) * 64,
                                   jb * 128: jb * 128 + 128],
                                qT[h * 64:(h + 1) * 64,
                                   icol + off: icol + 512],
                                start=True, stop=True,
                                tile_position=(h * 64, 0))
                    for h in range(2):
                        pt = ptp.tile([128, 1024], BF, tag="pt", name="ptd")
                        wtot = ws[0] + ws[1]
                        nc.scalar.activation(pt[:, 0:wtot],
                                             pss[h][:, 0:wtot], EXP)
                        for ui, rr in enumerate(rrs):
                            c0 = 0 if ui == 0 else ws[0]
                            nc.vector.tensor_mul(
                                pt[:, c0: c0 + 128], pt[:, c0: c0 + 128],
                                mask_sb[:, 384:512])
                            jb = b * 16 + 4 * I + rr
                            nc.tensor.matmul(
                                pv[h][:, 128 * rr: 512],
                                v_sb[:, 130 * jb + 65 * h:
                                     130 * jb + 65 * h + 65],
                                pt[:, c0: c0 + ws[ui]],
                                start=(I == 0 and rr == 0), stop=(rr == 3))
                # normalize out^T[d, i] by softmax denominator (pv row 64)
                for h in range(2):
                    pvb = smp.tile([65, 512], F32, tag="pvb", name="pvb")
                    nc.vector.tensor_copy(pvb[:], pv[h][:])
                    lr = smp.tile([1, 512], F32, tag="lr", name="lr")
                    nc.vector.tensor_copy(lr[:], pvb[64:65, :])
                    ell = smp.tile([1, 512], F32, tag="ell", name="ell")
                    nc.vector.reciprocal_approx_fast(ell[:], lr[:])
                    ellb = smp.tile([1, 512], BF, tag="ellb", name="ellb")
                    nc.vector.tensor_copy(ellb[:], ell[:])
                    pb = psQ.tile([64, 512], F32, tag="q", name="pb")
                    nc.tensor.matmul(pb[:], ones_sb[0:1, 0:64], ellb[0:1, :],
                                     start=True, stop=True)
                    rb = smp.tile([64, 512], BF, tag="rb", name="rb")
                    nc.vector.tensor_copy(rb[:], pb[:])
                    nc.vector.tensor_mul(
                        staged[h * 64:(h + 1) * 64, icol:icol + 512],
                        pvb[0:64, :], rb[:])

            # ---- exchange waves + projection ------------------------
            def exchange(wave, src0, width, ain, aout):
                # chunk j = staged[:, src0 + width*j : +width]
                nc.gpsimd.dma_start(
                    ain[:].rearrange("(c p) i -> p c i", p=128),
                    staged[:, src0: src0 + NCORES * width]
                    .rearrange("p (c i) -> p c i", c=NCORES))
                nc.gpsimd.collective_compute(
                    "AllToAll", mybir.AluOpType.bypass,
                    replica_groups=[list(range(NCORES))],
                    ins=[ain[:]], outs=[aout[:]])

            def load_po(po, aout, width):
                nc.sync.dma_start(
                    po[:].rearrange("p (c i) -> p c i", c=NCORES),
                    aout[:].rearrange("(c p) i -> p c i", p=128))

            def proj_tb(tb, po, width):
                # y rows tb*128.. from po (lhsT: [dims, 128 tokens])
                ci = (tb * 128) % width
                for co in range(2):
                    ps = psQ.tile([128, 512], F32, tag="q", name="psy")
                    for kk in range(NKT):
                        nc.tensor.matmul(
                            ps[:],
                            po[:, kk * width + ci: kk * width + ci + 128],
                            wp_sb[:, kk * C + co * 512:
                                  kk * C + co * 512 + 512],
                            start=(kk == 0), stop=(kk == NKT - 1))
                    ysb = yp.tile([128, 512], F32, tag="y", name="ysb")
                    nc.vector.tensor_add(
                        ysb[:], ps[:], bias_sb[:, co * 512:co * 512 + 512])
                    nc.sync.dma_start(
                        y_d[tb * 128:(tb + 1) * 128,
                            co * 512:(co + 1) * 512], ysb[:])

            for I in range(4):          # b0: qkv one step ahead of attn
                qkv_block(0, I, I + 1)
                if I < 3:
                    attn_block(0, I)
            qkv_block(1, 0, 1)
            attn_block(0, 3)
            attn_block(1, 0)
            qkv_block(1, 1, 2)
            exchange(0, 0, HTS, a2a_in[0], a2a_out[0])       # b0, hidden
            qkv_block(1, 2, 3)
            attn_block(1, 1)
            qkv_block(1, 3, 4)
            exchange(1, T, 128, a2a_in[1], a2a_out[1])       # b1 1st half
            load_po(po0, a2a_out[0], HTS)
            attn_block(1, 2)
            proj_tb(0, po0, HTS)
            load_po(po1a, a2a_out[1], 128)
            attn_block(1, 3)
            exchange(2, T + 1024, 128, a2a_in[2], a2a_out[2])  # b1 2nd half
            proj_tb(1, po0, HTS)
            proj_tb(2, po1a, 128)
            load_po(po1b, a2a_out[2], 128)
            proj_tb(3, po1b, 128)

    nc.compile()
    return nc


def _prep_inputs(x, w_qkv, w_proj, b_proj):
    bf = ml_dtypes.bfloat16
    xT = np.ascontiguousarray(x.reshape(G, C).T).astype(bf)
    wpT = np.ascontiguousarray(w_proj.T).astype(bf)
    bias = np.ascontiguousarray(
        np.broadcast_to(b_proj.astype(np.float32), (128, C)))
    mask = (np.arange(896)[None, :] - 384 >=
            np.arange(128)[:, None]).astype(bf)
    scale = np.float32(HS ** -0.5)
    in_maps = []
    for c in range(NCORES):
        h0, h1 = 2 * c, 2 * c + 1
        cols = []
        for part, sc in ((slice(0, 64), scale), (slice(64, 128), None),
                         (slice(128, 192), None)):
            for h in (h0, h1):
                w = w_qkv[h, part, :]
                if sc is not None:
                    w = w * sc
                cols.append(np.ascontiguousarray(w.T))
        wc = np.concatenate(cols, axis=1).astype(bf)   # [C, 384]
        in_maps.append({"xT": xT, "wqkv": wc, "wpT": wpT,
                        "bias": bias, "mask": mask})
    return in_maps


def _get_nc():
    global _CACHED
    if _CACHED is None:
        _CACHED = _build()
    return _CACHED


def run_on_cores(in_maps, **kwargs):
    nc = _get_nc()
    return bass_utils.run_bass_kernel_spmd(
        nc, in_maps, core_ids=list(range(NCORES)), **kwargs)


def kernel(x, w_qkv, w_proj, b_proj):
    in_maps = _prep_inputs(x, w_qkv, w_proj, b_proj)
    res = run_on_cores(in_maps)
    y = np.empty((B, T, C), dtype=np.float32)
    for c in range(NCORES):
        yc = res.results[c]["y"]
        y[0, HTS * c: HTS * (c + 1), :] = yc[0:256]
        y[1, 128 * c: 128 * (c + 1), :] = yc[256:384]
        y[1, 1024 + 128 * c: 1024 + 128 * (c + 1), :] = yc[384:512]
    return y


# revision 12
# speedup vs baseline: 1.0824x; 1.0824x over previous
"""Multihead causal attention block on 8 Trainium2 NeuronCores.

Sharding: tensor-parallel over heads (2 heads/core). Each core computes
qkv + attention for its heads over all tokens; two AllToAlls (one per
batch element, pipelined against attention compute) redistribute
attention outputs so each core holds all 1024 feature dims for two
256-token half-slices, where it runs the output projection locally.

Fixed problem shape: B=2, T=2048, C=1024, H=16, HS=64.
"""

import sys

sys.path.insert(0, "/opt/trn_rl_repo")

import numpy as np
import ml_dtypes

import concourse.bass as bass
import concourse.tile as tile
from concourse import bacc, mybir
from concourse import bass_utils

B, T, C = 2, 2048, 1024
H, HS = 16, 64
G = B * T              # 4096 global tokens (b-major)
NCORES = 8
NKT = C // 128         # 8 contraction tiles
HTS = T // NCORES      # 256-token half-slice per core per batch

dt = mybir.dt
BF = dt.bfloat16
F32 = dt.float32
EXP = mybir.ActivationFunctionType.Exp

_CACHED = None


def _build():
    nc = bacc.Bacc("TRN2", target_bir_lowering=False, debug=False,
                   num_devices=NCORES)

    xT_d = nc.dram_tensor("xT", [C, G], BF, kind="ExternalInput")
    wqkv_d = nc.dram_tensor("wqkv", [C, 384], BF, kind="ExternalInput")
    wpT_d = nc.dram_tensor("wpT", [C, C], BF, kind="ExternalInput")
    bias_d = nc.dram_tensor("bias", [128, C], F32, kind="ExternalInput")
    mask_d = nc.dram_tensor("mask", [128, 896], BF, kind="ExternalInput")
    y_d = nc.dram_tensor("y", [2 * HTS, C], F32, kind="ExternalOutput")

    a2a_shapes = [HTS, 128, 128]
    a2a_in = [nc.dram_tensor(f"a2a_in{w}", [NCORES * 128, a2a_shapes[w]], BF)
              for w in range(3)]
    a2a_out = [nc.dram_tensor(f"a2a_out{w}", [NCORES * 128, a2a_shapes[w]], BF)
               for w in range(3)]

    with tile.TileContext(nc) as tc:
        with tc.tile_pool(name="cst", bufs=1) as cst, \
             tc.tile_pool(name="pt", bufs=3) as ptp, \
             tc.tile_pool(name="sm", bufs=4) as smp, \
             tc.tile_pool(name="yp", bufs=3) as yp, \
             tc.tile_pool(name="psS", bufs=2, space="PSUM") as psS, \
             tc.tile_pool(name="psPV", bufs=2, space="PSUM") as psPV, \
             tc.tile_pool(name="psQ", bufs=2, space="PSUM") as psQ:

            # ---- constant loads -------------------------------------
            x_sb = cst.tile([128, NKT * G], BF)        # x^T c-tiles
            w_sb = cst.tile([128, NKT * 384], BF)      # per-head qkv weights
            mask_sb = cst.tile([128, 896], BF)
            bias_sb = cst.tile([128, C], F32)
            wp_sb = cst.tile([128, NKT * C], BF)       # w_proj^T c-tiles
            qT = cst.tile([128, G], BF)                # q^T (2 heads stacked)
            kT = cst.tile([128, G], BF)
            v_sb = cst.tile([128, 32 * 130], BF)       # v natural + ones col
            staged = cst.tile([128, G], BF)            # normalized out^T
            ones_sb = cst.tile([1, 64], BF)
            po0 = cst.tile([128, NCORES * 256], BF)    # b0 tokens 256j
            po1a = cst.tile([128, NCORES * 128], BF)   # b1 tokens 128j
            po1b = cst.tile([128, NCORES * 128], BF)   # b1 tokens 1024+128j

            nc.sync.dma_start(
                w_sb[:].rearrange("p (k n) -> p k n", k=NKT),
                wqkv_d[:].rearrange("(k p) n -> p k n", p=128))
            # x in priority waves of single wide DMAs (HWDGE fans each
            # across all 16 SDMA engines); earliest tokens first
            x_sb_v = x_sb[:].rearrange("p (k g) -> p k g", k=NKT)
            xT_v = xT_d[:].rearrange("(k p) g -> p k g", p=128)
            for w0, w1 in ((0, 512), (512, 1024), (1024, T),
                           (T, 3 * 1024), (3 * 1024, G)):
                nc.sync.dma_start(x_sb_v[:, :, w0:w1], xT_v[:, :, w0:w1])
            nc.sync.dma_start(mask_sb[:], mask_d[:])
            nc.sync.dma_start(bias_sb[:], bias_d[:])
            nc.sync.dma_start(
                wp_sb[:].rearrange("p (k n) -> p k n", k=NKT),
                wpT_d[:].rearrange("(k p) n -> p k n", p=128))

            nc.vector.memset(ones_sb[:], 1.0)
            # ones column of v_aug: offsets 64 + 65*m
            v_ones = v_sb[:].rearrange("p (m o) -> p m o", o=65)[:, :, 64:65]
            nc.vector.memset(v_ones, 1.0)

            # ---- qkv projections ------------------------------------
            def qkv_block(b, tb0=0, tb1=4):
                for tb in range(tb0, tb1):
                    gt = b * T + tb * 512
                    for part in range(2):      # 0=q pair, 1=k pair
                        ps = psQ.tile([128, 512], F32, tag="q", name="psqk")
                        for kk in range(NKT):
                            nc.tensor.matmul(
                                ps[:],
                                w_sb[:, kk * 384 + part * 128:
                                     kk * 384 + part * 128 + 128],
                                x_sb[:, kk * G + gt: kk * G + gt + 512],
                                start=(kk == 0), stop=(kk == NKT - 1))
                        dst = qT if part == 0 else kT
                        nc.vector.tensor_copy(dst[:, gt:gt + 512], ps[:])
                    for ts in range(4):        # v in natural layout
                        g0 = gt + 128 * ts
                        jb = g0 // 128
                        ps = psQ.tile([128, 128], F32, tag="q", name="psv")
                        for kk in range(NKT):
                            nc.tensor.matmul(
                                ps[:],
                                x_sb[:, kk * G + g0: kk * G + g0 + 128],
                                w_sb[:, kk * 384 + 256: kk * 384 + 384],
                                start=(kk == 0), stop=(kk == NKT - 1))
                        dst = v_sb[:, 130 * jb: 130 * jb + 130] \
                            .rearrange("p (h o) -> p h o", o=65)[:, :, 0:64]
                        nc.vector.tensor_copy(
                            dst, ps[:].rearrange("p (h d) -> p h d", d=64))

            # ---- attention for one (b, I) i-block of 512 ------------
            def attn_block(b, I):
                icol = (b * 4 + I) * 512
                pv = [psPV.tile([65, 512], F32, tag="pv", name=f"pv{b}{I}{hh}")
                      for hh in range(2)]
                # off-diagonal j-blocks, chunks of 2 (no mask needed)
                for cc in range(2 * I):
                    pss = [psS.tile([128, 1024], F32, tag="s",
                                    name=f"pss{hh}") for hh in range(2)]
                    for u in range(2):
                        for h in range(2):
                            jb = b * 16 + 2 * cc + u
                            nc.tensor.matmul(
                                pss[h][:, u * 512:(u + 1) * 512],
                                kT[h * 64:(h + 1) * 64,
                                   jb * 128: jb * 128 + 128],
                                qT[h * 64:(h + 1) * 64, icol: icol + 512],
                                start=True, stop=True,
                                tile_position=(h * 64, 0))
                    for h in range(2):
                        pt = ptp.tile([128, 1024], BF, tag="pt", name="pt")
                        nc.scalar.activation(pt[:], pss[h][:], EXP)
                        for u in range(2):
                            jb = b * 16 + 2 * cc + u
                            nc.tensor.matmul(
                                pv[h][:],
                                v_sb[:, 130 * jb + 65 * h:
                                     130 * jb + 65 * h + 65],
                                pt[:, u * 512:(u + 1) * 512],
                                start=(cc == 0 and u == 0), stop=False)
                # diagonal j-blocks: shrink to valid columns, batch rr
                # pairs into one psum tile / one exp, triangle mask
                for rp in range(2):            # rr pair: (0,1) or (2,3)
                    rrs = (2 * rp, 2 * rp + 1)
                    ws = [512 - 128 * rr for rr in rrs]
                    pss = [psS.tile([128, 1024], F32, tag="s",
                                    name=f"psd{hh}") for hh in range(2)]
                    for ui, rr in enumerate(rrs):
                        off = 128 * rr
                        c0 = 0 if ui == 0 else ws[0]
                        for h in range(2):
                            jb = b * 16 + 4 * I + rr
                            nc.tensor.matmul(
                                pss[h][:, c0: c0 + ws[ui]],
                                kT[h * 64:(h + 1) * 64,
                                   jb * 128: jb * 128 + 128],
                                qT[h * 64:(h + 1) * 64,
                                   icol + off: icol + 512],
                                start=True, stop=True,
                                tile_position=(h * 64, 0))
                    for h in range(2):
                        pt = ptp.tile([128, 1024], BF, tag="pt", name="ptd")
                        wtot = ws[0] + ws[1]
                        nc.scalar.activation(pt[:, 0:wtot],
                                             pss[h][:, 0:wtot], EXP)
                        for ui, rr in enumerate(rrs):
                            c0 = 0 if ui == 0 else ws[0]
                            nc.vector.tensor_mul(
                                pt[:, c0: c0 + 128], pt[:, c0: c0 + 128],
                                mask_sb[:, 384:512])
                            jb = b * 16 + 4 * I + rr
                            nc.tensor.matmul(
                                pv[h][:, 128 * rr: 512],
                                v_sb[:, 130 * jb + 65 * h:
                                     130 * jb + 65 * h + 65],
                                pt[:, c0: c0 + ws[ui]],
                                start=(I == 0 and rr == 0), stop=(rr == 3))
                # normalize out^T[d, i] by softmax denominator (pv row 64)
                for h in range(2):
                    pvb = smp.tile([65, 512], F32, tag="pvb", name="pvb")
                    nc.vector.tensor_copy(pvb[:], pv[h][:])
                    lr = smp.tile([1, 512], F32, tag="lr", name="lr")
                    nc.vector.tensor_copy(lr[:], pvb[64:65, :])
                    ell = smp.tile([1, 512], F32, tag="ell", name="ell")
                    nc.vector.reciprocal_approx_fast(ell[:], lr[:])
                    ellb = smp.tile([1, 512], BF, tag="ellb", name="ellb")
                    nc.vector.tensor_copy(ellb[:], ell[:])
                    pb = psQ.tile([64, 512], F32, tag="q", name="pb")
                    nc.tensor.matmul(pb[:], ones_sb[0:1, 0:64], ellb[0:1, :],
                                     start=True, stop=True)
                    rb = smp.tile([64, 512], BF, tag="rb", name="rb")
                    nc.vector.tensor_copy(rb[:], pb[:])
                    nc.vector.tensor_mul(
                        staged[h * 64:(h + 1) * 64, icol:icol + 512],
                        pvb[0:64, :], rb[:])

            # ---- exchange waves + projection ------------------------
            def exchange(wave, src0, width, ain, aout):
                # chunk j = staged[:, src0 + width*j : +width]
                nc.gpsimd.dma_start(
                    ain[:].rearrange("(c p) i -> p c i", p=128),
                    staged[:, src0: src0 + NCORES * width]
                    .rearrange("p (c i) -> p c i", c=NCORES))
                nc.gpsimd.collective_compute(
                    "AllToAll", mybir.AluOpType.bypass,
                    replica_groups=[list(range(NCORES))],
                    ins=[ain[:]], outs=[aout[:]])

            def load_po(po, aout, width):
                nc.sync.dma_start(
                    po[:].rearrange("p (c i) -> p c i", c=NCORES),
                    aout[:].rearrange("(c p) i -> p c i", p=128))

            def proj_tb(tb, po, width):
                # y rows tb*128.. from po (lhsT: [dims, 128 tokens])
                ci = (tb * 128) % width
                for co in range(2):
                    ps = psQ.tile([128, 512], F32, tag="q", name="psy")
                    for kk in range(NKT):
                        nc.tensor.matmul(
                            ps[:],
                            po[:, kk * width + ci: kk * width + ci + 128],
                            wp_sb[:, kk * C + co * 512:
                                  kk * C + co * 512 + 512],
                            start=(kk == 0), stop=(kk == NKT - 1))
                    ysb = yp.tile([128, 512], F32, tag="y", name="ysb")
                    nc.vector.tensor_add(
                        ysb[:], ps[:], bias_sb[:, co * 512:co * 512 + 512])
                    nc.sync.dma_start(
                        y_d[tb * 128:(tb + 1) * 128,
                            co * 512:(co + 1) * 512], ysb[:])

            for I in range(4):          # b0: qkv one step ahead of attn
                qkv_block(0, I, I + 1)
                if I < 3:
                    attn_block(0, I)
            qkv_block(1, 0, 1)
            attn_block(0, 3)
            attn_block(1, 0)
            qkv_block(1, 1, 2)
            exchange(0, 0, HTS, a2a_in[0], a2a_out[0])       # b0, hidden
            qkv_block(1, 2, 3)
            attn_block(1, 1)
            qkv_block(1, 3, 4)
            exchange(1, T, 128, a2a_in[1], a2a_out[1])       # b1 1st half
            load_po(po0, a2a_out[0], HTS)
            attn_block(1, 2)
            proj_tb(0, po0, HTS)
            load_po(po1a, a2a_out[1], 128)
            attn_block(1, 3)
            exchange(2, T + 1024, 128, a2a_in[2], a2a_out[2])  # b1 2nd half
            proj_tb(1, po0, HTS)
            proj_tb(2, po1a, 128)
            load_po(po1b, a2a_out[2], 128)
            proj_tb(3, po1b, 128)



    nc.compile()
    return nc


def _prep_inputs(x, w_qkv, w_proj, b_proj):
    bf = ml_dtypes.bfloat16
    xT = np.ascontiguousarray(x.reshape(G, C).T).astype(bf)
    wpT = np.ascontiguousarray(w_proj.T).astype(bf)
    bias = np.ascontiguousarray(
        np.broadcast_to(b_proj.astype(np.float32), (128, C)))
    mask = (np.arange(896)[None, :] - 384 >=
            np.arange(128)[:, None]).astype(bf)
    scale = np.float32(HS ** -0.5)
    in_maps = []
    for c in range(NCORES):
        h0, h1 = 2 * c, 2 * c + 1
        cols = []
        for part, sc in ((slice(0, 64), scale), (slice(64, 128), None),
                         (slice(128, 192), None)):
            for h in (h0, h1):
                w = w_qkv[h, part, :]
                if sc is not None:
                    w = w * sc
                cols.append(np.ascontiguousarray(w.T))
        wc = np.concatenate(cols, axis=1).astype(bf)   # [C, 384]
        in_maps.append({"xT": xT, "wqkv": wc, "wpT": wpT,
                        "bias": bias, "mask": mask})
    return in_maps


def _get_nc():
    global _CACHED
    if _CACHED is None:
        _CACHED = _build()
    return _CACHED


def run_on_cores(in_maps, **kwargs):
    nc = _get_nc()
    return bass_utils.run_bass_kernel_spmd(
        nc, in_maps, core_ids=list(range(NCORES)), **kwargs)


def kernel(x, w_qkv, w_proj, b_proj):
    in_maps = _prep_inputs(x, w_qkv, w_proj, b_proj)
    res = run_on_cores(in_maps)
    y = np.empty((B, T, C), dtype=np.float32)
    for c in range(NCORES):
        yc = res.results[c]["y"]
        y[0, HTS * c: HTS * (c + 1), :] = yc[0:256]
        y[1, 128 * c: 128 * (c + 1), :] = yc[256:384]
        y[1, 1024 + 128 * c: 1024 + 128 * (c + 1), :] = yc[384:512]
    return y


# revision 13
# speedup vs baseline: 1.1576x; 1.0695x over previous
"""Multihead causal attention block on 8 Trainium2 NeuronCores.

Sharding: tensor-parallel over heads (2 heads/core). Each core computes
qkv + attention for its heads over all tokens; two AllToAlls (one per
batch element, pipelined against attention compute) redistribute
attention outputs so each core holds all 1024 feature dims for two
256-token half-slices, where it runs the output projection locally.

Fixed problem shape: B=2, T=2048, C=1024, H=16, HS=64.
"""

import sys

sys.path.insert(0, "/opt/trn_rl_repo")

import numpy as np
import ml_dtypes

import concourse.bass as bass
import concourse.tile as tile
from concourse import bacc, mybir
from concourse import bass_utils

B, T, C = 2, 2048, 1024
H, HS = 16, 64
G = B * T              # 4096 global tokens (b-major)
NCORES = 8
NKT = C // 128         # 8 contraction tiles
HTS = T // NCORES      # 256-token half-slice per core per batch

dt = mybir.dt
BF = dt.bfloat16
F32 = dt.float32
EXP = mybir.ActivationFunctionType.Exp

_CACHED = None


def _build():
    nc = bacc.Bacc("TRN2", target_bir_lowering=False, debug=False,
                   num_devices=NCORES)

    xT_d = nc.dram_tensor("xT", [C, G], BF, kind="ExternalInput")
    wqkv_d = nc.dram_tensor("wqkv", [C, 384], BF, kind="ExternalInput")
    wpT_d = nc.dram_tensor("wpT", [C, C], BF, kind="ExternalInput")
    bias_d = nc.dram_tensor("bias", [128, C], F32, kind="ExternalInput")
    mask_d = nc.dram_tensor("mask", [128, 896], BF, kind="ExternalInput")
    y_d = nc.dram_tensor("y", [384, C], F32, kind="ExternalOutput")
    y2_d = nc.dram_tensor("y2", [1024, C], F32, kind="ExternalOutput")
    wpo_d = nc.dram_tensor("wpo", [128, C], BF, kind="ExternalInput")

    a2a_shapes = [HTS, 128]
    a2a_in = [nc.dram_tensor(f"a2a_in{w}", [NCORES * 128, a2a_shapes[w]], BF)
              for w in range(2)]
    a2a_out = [nc.dram_tensor(f"a2a_out{w}", [NCORES * 128, a2a_shapes[w]], BF)
               for w in range(2)]

    with tile.TileContext(nc) as tc:
        with tc.tile_pool(name="cst", bufs=1) as cst, \
             tc.tile_pool(name="pt", bufs=3) as ptp, \
             tc.tile_pool(name="sm", bufs=4) as smp, \
             tc.tile_pool(name="yp", bufs=3) as yp, \
             tc.tile_pool(name="psS", bufs=2, space="PSUM") as psS, \
             tc.tile_pool(name="psPV", bufs=2, space="PSUM") as psPV, \
             tc.tile_pool(name="psQ", bufs=2, space="PSUM") as psQ:

            # ---- constant loads -------------------------------------
            x_sb = cst.tile([128, NKT * G], BF)        # x^T c-tiles
            w_sb = cst.tile([128, NKT * 384], BF)      # per-head qkv weights
            mask_sb = cst.tile([128, 896], BF)
            bias_sb = cst.tile([128, C], F32)
            wp_sb = cst.tile([128, NKT * C], BF)       # w_proj^T c-tiles
            qT = cst.tile([128, G], BF)                # q^T (2 heads stacked)
            kT = cst.tile([128, G], BF)
            v_sb = cst.tile([128, 32 * 130], BF)       # v natural + ones col
            staged = cst.tile([128, G], BF)            # normalized out^T
            ones_sb = cst.tile([1, 64], BF)
            po0 = cst.tile([128, NCORES * 256], BF)    # b0 tokens 256j
            po1a = cst.tile([128, NCORES * 128], BF)   # b1 tokens 128j
            wpo_sb = cst.tile([128, C], BF)            # own rows of w_proj^T

            nc.sync.dma_start(
                w_sb[:].rearrange("p (k n) -> p k n", k=NKT),
                wqkv_d[:].rearrange("(k p) n -> p k n", p=128))
            # x in priority waves of single wide DMAs (HWDGE fans each
            # across all 16 SDMA engines); earliest tokens first
            x_sb_v = x_sb[:].rearrange("p (k g) -> p k g", k=NKT)
            xT_v = xT_d[:].rearrange("(k p) g -> p k g", p=128)
            for w0, w1 in ((0, 512), (512, 1024), (1024, T),
                           (T, 3 * 1024), (3 * 1024, G)):
                nc.sync.dma_start(x_sb_v[:, :, w0:w1], xT_v[:, :, w0:w1])
            nc.sync.dma_start(mask_sb[:], mask_d[:])
            nc.sync.dma_start(wpo_sb[:], wpo_d[:])
            nc.sync.dma_start(bias_sb[:], bias_d[:])
            nc.sync.dma_start(
                wp_sb[:].rearrange("p (k n) -> p k n", k=NKT),
                wpT_d[:].rearrange("(k p) n -> p k n", p=128))

            nc.vector.memset(ones_sb[:], 1.0)
            # ones column of v_aug: offsets 64 + 65*m
            v_ones = v_sb[:].rearrange("p (m o) -> p m o", o=65)[:, :, 64:65]
            nc.vector.memset(v_ones, 1.0)

            # ---- qkv projections ------------------------------------
            def qkv_block(b, tb0=0, tb1=4):
                for tb in range(tb0, tb1):
                    gt = b * T + tb * 512
                    for part in range(2):      # 0=q pair, 1=k pair
                        ps = psQ.tile([128, 512], F32, tag="q", name="psqk")
                        for kk in range(NKT):
                            nc.tensor.matmul(
                                ps[:],
                                w_sb[:, kk * 384 + part * 128:
                                     kk * 384 + part * 128 + 128],
                                x_sb[:, kk * G + gt: kk * G + gt + 512],
                                start=(kk == 0), stop=(kk == NKT - 1))
                        dst = qT if part == 0 else kT
                        nc.vector.tensor_copy(dst[:, gt:gt + 512], ps[:])
                    for ts in range(4):        # v in natural layout
                        g0 = gt + 128 * ts
                        jb = g0 // 128
                        ps = psQ.tile([128, 128], F32, tag="q", name="psv")
                        for kk in range(NKT):
                            nc.tensor.matmul(
                                ps[:],
                                x_sb[:, kk * G + g0: kk * G + g0 + 128],
                                w_sb[:, kk * 384 + 256: kk * 384 + 384],
                                start=(kk == 0), stop=(kk == NKT - 1))
                        dst = v_sb[:, 130 * jb: 130 * jb + 130] \
                            .rearrange("p (h o) -> p h o", o=65)[:, :, 0:64]
                        nc.vector.tensor_copy(
                            dst, ps[:].rearrange("p (h d) -> p h d", d=64))

            # ---- attention for one (b, I) i-block of 512 ------------
            def attn_block(b, I):
                icol = (b * 4 + I) * 512
                pv = [psPV.tile([65, 512], F32, tag="pv", name=f"pv{b}{I}{hh}")
                      for hh in range(2)]
                # off-diagonal j-blocks, chunks of 2 (no mask needed)
                for cc in range(2 * I):
                    pss = [psS.tile([128, 1024], F32, tag="s",
                                    name=f"pss{hh}") for hh in range(2)]
                    for u in range(2):
                        for h in range(2):
                            jb = b * 16 + 2 * cc + u
                            nc.tensor.matmul(
                                pss[h][:, u * 512:(u + 1) * 512],
                                kT[h * 64:(h + 1) * 64,
                                   jb * 128: jb * 128 + 128],
                                qT[h * 64:(h + 1) * 64, icol: icol + 512],
                                start=True, stop=True,
                                tile_position=(h * 64, 0))
                    for h in range(2):
                        pt = ptp.tile([128, 1024], BF, tag="pt", name="pt")
                        nc.scalar.activation(pt[:], pss[h][:], EXP)
                        for u in range(2):
                            jb = b * 16 + 2 * cc + u
                            nc.tensor.matmul(
                                pv[h][:],
                                v_sb[:, 130 * jb + 65 * h:
                                     130 * jb + 65 * h + 65],
                                pt[:, u * 512:(u + 1) * 512],
                                start=(cc == 0 and u == 0), stop=False)
                # diagonal j-blocks: shrink to valid columns, batch rr
                # pairs into one psum tile / one exp, triangle mask
                for rp in range(2):            # rr pair: (0,1) or (2,3)
                    rrs = (2 * rp, 2 * rp + 1)
                    ws = [512 - 128 * rr for rr in rrs]
                    pss = [psS.tile([128, 1024], F32, tag="s",
                                    name=f"psd{hh}") for hh in range(2)]
                    for ui, rr in enumerate(rrs):
                        off = 128 * rr
                        c0 = 0 if ui == 0 else ws[0]
                        for h in range(2):
                            jb = b * 16 + 4 * I + rr
                            nc.tensor.matmul(
                                pss[h][:, c0: c0 + ws[ui]],
                                kT[h * 64:(h + 1) * 64,
                                   jb * 128: jb * 128 + 128],
                                qT[h * 64:(h + 1) * 64,
                                   icol + off: icol + 512],
                                start=True, stop=True,
                                tile_position=(h * 64, 0))
                    for h in range(2):
                        pt = ptp.tile([128, 1024], BF, tag="pt", name="ptd")
                        wtot = ws[0] + ws[1]
                        nc.scalar.activation(pt[:, 0:wtot],
                                             pss[h][:, 0:wtot], EXP)
                        for ui, rr in enumerate(rrs):
                            c0 = 0 if ui == 0 else ws[0]
                            nc.vector.tensor_mul(
                                pt[:, c0: c0 + 128], pt[:, c0: c0 + 128],
                                mask_sb[:, 384:512])
                            jb = b * 16 + 4 * I + rr
                            nc.tensor.matmul(
                                pv[h][:, 128 * rr: 512],
                                v_sb[:, 130 * jb + 65 * h:
                                     130 * jb + 65 * h + 65],
                                pt[:, c0: c0 + ws[ui]],
                                start=(I == 0 and rr == 0), stop=(rr == 3))
                # normalize out^T[d, i] by softmax denominator (pv row 64)
                for h in range(2):
                    pvb = smp.tile([65, 512], F32, tag="pvb", name="pvb")
                    nc.vector.tensor_copy(pvb[:], pv[h][:])
                    lr = smp.tile([1, 512], F32, tag="lr", name="lr")
                    nc.vector.tensor_copy(lr[:], pvb[64:65, :])
                    ell = smp.tile([1, 512], F32, tag="ell", name="ell")
                    nc.vector.reciprocal_approx_fast(ell[:], lr[:])
                    ellb = smp.tile([1, 512], BF, tag="ellb", name="ellb")
                    nc.vector.tensor_copy(ellb[:], ell[:])
                    pb = psQ.tile([64, 512], F32, tag="q", name="pb")
                    nc.tensor.matmul(pb[:], ones_sb[0:1, 0:64], ellb[0:1, :],
                                     start=True, stop=True)
                    rb = smp.tile([64, 512], BF, tag="rb", name="rb")
                    nc.vector.tensor_copy(rb[:], pb[:])
                    nc.vector.tensor_mul(
                        staged[h * 64:(h + 1) * 64, icol:icol + 512],
                        pvb[0:64, :], rb[:])

            # ---- exchange waves + projection ------------------------
            def exchange(wave, src0, width, ain, aout):
                # chunk j = staged[:, src0 + width*j : +width]
                nc.gpsimd.dma_start(
                    ain[:].rearrange("(c p) i -> p c i", p=128),
                    staged[:, src0: src0 + NCORES * width]
                    .rearrange("p (c i) -> p c i", c=NCORES))
                nc.gpsimd.collective_compute(
                    "AllToAll", mybir.AluOpType.bypass,
                    replica_groups=[list(range(NCORES))],
                    ins=[ain[:]], outs=[aout[:]])

            def load_po(po, aout, width):
                nc.sync.dma_start(
                    po[:].rearrange("p (c i) -> p c i", c=NCORES),
                    aout[:].rearrange("(c p) i -> p c i", p=128))

            def proj_tb(tb, po, width):
                # y rows tb*128.. from po (lhsT: [dims, 128 tokens])
                ci = (tb * 128) % width
                for co in range(2):
                    ps = psQ.tile([128, 512], F32, tag="q", name="psy")
                    for kk in range(NKT):
                        nc.tensor.matmul(
                            ps[:],
                            po[:, kk * width + ci: kk * width + ci + 128],
                            wp_sb[:, kk * C + co * 512:
                                  kk * C + co * 512 + 512],
                            start=(kk == 0), stop=(kk == NKT - 1))
                    ysb = yp.tile([128, 512], F32, tag="y", name="ysb")
                    nc.vector.tensor_add(
                        ysb[:], ps[:], bias_sb[:, co * 512:co * 512 + 512])
                    nc.sync.dma_start(
                        y_d[tb * 128:(tb + 1) * 128,
                            co * 512:(co + 1) * 512], ysb[:])

            def pproj(k):
                # tokens 3072+512k .. +512 of staged -> y2 rows 512k..
                for tb in range(4):
                    g0 = 3072 + 512 * k + 128 * tb
                    for co in range(2):
                        ps = psQ.tile([128, 512], F32, tag="q", name="psp")
                        nc.tensor.matmul(
                            ps[:], staged[:, g0: g0 + 128],
                            wpo_sb[:, co * 512: co * 512 + 512],
                            start=True, stop=True)
                        ysb = yp.tile([128, 512], F32, tag="y", name="y2sb")
                        nc.vector.tensor_copy(ysb[:], ps[:])
                        nc.sync.dma_start(
                            y2_d[512 * k + 128 * tb: 512 * k + 128 * tb + 128,
                                 co * 512:(co + 1) * 512], ysb[:])

            for I in range(4):          # b0: qkv one step ahead of attn
                qkv_block(0, I, I + 1)
                if I < 3:
                    attn_block(0, I)
            qkv_block(1, 0, 1)
            attn_block(0, 3)
            attn_block(1, 0)
            qkv_block(1, 1, 2)
            exchange(0, 0, HTS, a2a_in[0], a2a_out[0])       # b0, hidden
            qkv_block(1, 2, 3)
            attn_block(1, 1)
            qkv_block(1, 3, 4)
            exchange(1, T, 128, a2a_in[1], a2a_out[1])       # b1 1st half
            load_po(po0, a2a_out[0], HTS)
            attn_block(1, 2)
            proj_tb(0, po0, HTS)
            proj_tb(1, po0, HTS)
            pproj(0)               # tokens 1024-1535, local partial
            load_po(po1a, a2a_out[1], 128)
            proj_tb(2, po1a, 128)
            attn_block(1, 3)
            pproj(1)               # tokens 1536-2047, local partial


    nc.compile()
    return nc


def _prep_inputs(x, w_qkv, w_proj, b_proj):
    bf = ml_dtypes.bfloat16
    xT = np.ascontiguousarray(x.reshape(G, C).T).astype(bf)
    wpT = np.ascontiguousarray(w_proj.T).astype(bf)
    bias = np.ascontiguousarray(
        np.broadcast_to(b_proj.astype(np.float32), (128, C)))
    mask = (np.arange(896)[None, :] - 384 >=
            np.arange(128)[:, None]).astype(bf)
    scale = np.float32(HS ** -0.5)
    in_maps = []
    for c in range(NCORES):
        h0, h1 = 2 * c, 2 * c + 1
        cols = []
        for part, sc in ((slice(0, 64), scale), (slice(64, 128), None),
                         (slice(128, 192), None)):
            for h in (h0, h1):
                w = w_qkv[h, part, :]
                if sc is not None:
                    w = w * sc
                cols.append(np.ascontiguousarray(w.T))
        wc = np.concatenate(cols, axis=1).astype(bf)   # [C, 384]
        wpo = np.ascontiguousarray(wpT[128 * c:128 * (c + 1), :]).astype(bf)
        in_maps.append({"xT": xT, "wqkv": wc, "wpT": wpT,
                        "bias": bias, "mask": mask, "wpo": wpo})
    return in_maps


def _get_nc():
    global _CACHED
    if _CACHED is None:
        _CACHED = _build()
    return _CACHED


def run_on_cores(in_maps, **kwargs):
    nc = _get_nc()
    return bass_utils.run_bass_kernel_spmd(
        nc, in_maps, core_ids=list(range(NCORES)), **kwargs)


def kernel(x, w_qkv, w_proj, b_proj):
    in_maps = _prep_inputs(x, w_qkv, w_proj, b_proj)
    res = run_on_cores(in_maps)
    y = np.empty((B, T, C), dtype=np.float32)
    acc = None
    for c in range(NCORES):
        yc = res.results[c]["y"]
        y[0, HTS * c: HTS * (c + 1), :] = yc[0:256]
        y[1, 128 * c: 128 * (c + 1), :] = yc[256:384]
        y2 = res.results[c]["y2"]
        acc = y2 if acc is None else acc + y2
    y[1, 1024:2048, :] = acc + b_proj.astype(np.float32)[None, :]
    return y


# revision 15
# speedup vs baseline: 1.1948x; 1.0321x over previous
"""Multihead causal attention block on 8 Trainium2 NeuronCores.

Sharding: tensor-parallel over heads (2 heads/core). Each core computes
qkv + attention for its heads over all tokens; two AllToAlls (one per
batch element, pipelined against attention compute) redistribute
attention outputs so each core holds all 1024 feature dims for two
256-token half-slices, where it runs the output projection locally.

Fixed problem shape: B=2, T=2048, C=1024, H=16, HS=64.
"""

import sys

sys.path.insert(0, "/opt/trn_rl_repo")

import numpy as np
import ml_dtypes

import concourse.bass as bass
import concourse.tile as tile
from concourse import bacc, mybir
from concourse import bass_utils

B, T, C = 2, 2048, 1024
H, HS = 16, 64
G = B * T              # 4096 global tokens (b-major)
NCORES = 8
NKT = C // 128         # 8 contraction tiles
HTS = T // NCORES      # 256-token half-slice per core per batch

dt = mybir.dt
BF = dt.bfloat16
F32 = dt.float32
EXP = mybir.ActivationFunctionType.Exp

_CACHED = None


def _build():
    nc = bacc.Bacc("TRN2", target_bir_lowering=False, debug=False,
                   num_devices=NCORES)

    xT_d = nc.dram_tensor("xT", [C, G], BF, kind="ExternalInput")
    wqkv_d = nc.dram_tensor("wqkv", [C, 384], BF, kind="ExternalInput")
    wpT_d = nc.dram_tensor("wpT", [C, C], BF, kind="ExternalInput")
    bias_d = nc.dram_tensor("bias", [128, C], F32, kind="ExternalInput")
    mask_d = nc.dram_tensor("mask", [128, 896], BF, kind="ExternalInput")
    y_d = nc.dram_tensor("y", [384, C], F32, kind="ExternalOutput")
    y2_d = nc.dram_tensor("y2", [1024, C], F32, kind="ExternalOutput")
    wpo_d = nc.dram_tensor("wpo", [128, C], BF, kind="ExternalInput")

    a2a_shapes = [HTS, 128]
    a2a_in = [nc.dram_tensor(f"a2a_in{w}", [NCORES * 128, a2a_shapes[w]], BF)
              for w in range(2)]
    a2a_out = [nc.dram_tensor(f"a2a_out{w}", [NCORES * 128, a2a_shapes[w]], BF)
               for w in range(2)]

    with tile.TileContext(nc) as tc:
        with tc.tile_pool(name="cst", bufs=1) as cst, \
             tc.tile_pool(name="pt", bufs=3) as ptp, \
             tc.tile_pool(name="sm", bufs=4) as smp, \
             tc.tile_pool(name="yp", bufs=3) as yp, \
             tc.tile_pool(name="psS", bufs=2, space="PSUM") as psS, \
             tc.tile_pool(name="psPV", bufs=2, space="PSUM") as psPV, \
             tc.tile_pool(name="psQ", bufs=2, space="PSUM") as psQ:

            # ---- constant loads -------------------------------------
            x_sb = cst.tile([128, NKT * G], BF)        # x^T c-tiles
            w_sb = cst.tile([128, NKT * 384], BF)      # per-head qkv weights
            mask_sb = cst.tile([128, 896], BF)
            bias_sb = cst.tile([128, C], F32)
            wp_sb = cst.tile([128, NKT * C], BF)       # w_proj^T c-tiles
            qT = cst.tile([128, G], BF)                # q^T (2 heads stacked)
            kT = cst.tile([128, G], BF)
            v_sb = cst.tile([128, 32 * 130], BF)       # v natural + ones col
            staged = cst.tile([128, G], BF)            # normalized out^T
            ones_sb = cst.tile([1, 64], BF)
            po0 = cst.tile([128, NCORES * 256], BF)    # b0 tokens 256j
            po1a = cst.tile([128, NCORES * 128], BF)   # b1 tokens 128j
            wpo_sb = cst.tile([128, C], BF)            # own rows of w_proj^T

            nc.sync.dma_start(
                w_sb[:].rearrange("p (k n) -> p k n", k=NKT),
                wqkv_d[:].rearrange("(k p) n -> p k n", p=128))
            # x in priority waves of single wide DMAs (HWDGE fans each
            # across all 16 SDMA engines); earliest tokens first
            x_sb_v = x_sb[:].rearrange("p (k g) -> p k g", k=NKT)
            xT_v = xT_d[:].rearrange("(k p) g -> p k g", p=128)
            for w0, w1 in ((0, 512), (512, 1024), (1024, T),
                           (T, 3 * 1024), (3 * 1024, G)):
                nc.sync.dma_start(x_sb_v[:, :, w0:w1], xT_v[:, :, w0:w1])
            nc.sync.dma_start(mask_sb[:], mask_d[:])
            nc.sync.dma_start(wpo_sb[:], wpo_d[:])
            nc.sync.dma_start(bias_sb[:], bias_d[:])
            nc.sync.dma_start(
                wp_sb[:].rearrange("p (k n) -> p k n", k=NKT),
                wpT_d[:].rearrange("(k p) n -> p k n", p=128))

            nc.vector.memset(ones_sb[:], 1.0)
            # ones column of v_aug: offsets 64 + 65*m
            v_ones = v_sb[:].rearrange("p (m o) -> p m o", o=65)[:, :, 64:65]
            nc.vector.memset(v_ones, 1.0)

            # ---- qkv projections ------------------------------------
            def qkv_block(b, tb0=0, tb1=4):
                for tb in range(tb0, tb1):
                    gt = b * T + tb * 512
                    for part in range(2):      # 0=q pair, 1=k pair
                        ps = psQ.tile([128, 512], F32, tag="q", name="psqk")
                        for kk in range(NKT):
                            nc.tensor.matmul(
                                ps[:],
                                w_sb[:, kk * 384 + part * 128:
                                     kk * 384 + part * 128 + 128],
                                x_sb[:, kk * G + gt: kk * G + gt + 512],
                                start=(kk == 0), stop=(kk == NKT - 1))
                        dst = qT if part == 0 else kT
                        nc.vector.tensor_copy(dst[:, gt:gt + 512], ps[:])
                    for ts in range(4):        # v in natural layout
                        g0 = gt + 128 * ts
                        jb = g0 // 128
                        ps = psQ.tile([128, 128], F32, tag="q", name="psv")
                        for kk in range(NKT):
                            nc.tensor.matmul(
                                ps[:],
                                x_sb[:, kk * G + g0: kk * G + g0 + 128],
                                w_sb[:, kk * 384 + 256: kk * 384 + 384],
                                start=(kk == 0), stop=(kk == NKT - 1))
                        dst = v_sb[:, 130 * jb: 130 * jb + 130] \
                            .rearrange("p (h o) -> p h o", o=65)[:, :, 0:64]
                        nc.vector.tensor_copy(
                            dst, ps[:].rearrange("p (h d) -> p h d", d=64))

            # ---- attention for one (b, I) i-block of 512 ------------
            def attn_block(b, I):
                icol = (b * 4 + I) * 512
                pv = [psPV.tile([65, 512], F32, tag="pv", name=f"pv{b}{I}{hh}")
                      for hh in range(2)]
                # off-diagonal j-blocks, chunks of 2 (no mask needed)
                for cc in range(2 * I):
                    pss = [psS.tile([128, 1024], F32, tag="s",
                                    name=f"pss{hh}") for hh in range(2)]
                    for u in range(2):
                        for h in range(2):
                            jb = b * 16 + 2 * cc + u
                            nc.tensor.matmul(
                                pss[h][:, u * 512:(u + 1) * 512],
                                kT[h * 64:(h + 1) * 64,
                                   jb * 128: jb * 128 + 128],
                                qT[h * 64:(h + 1) * 64, icol: icol + 512],
                                start=True, stop=True,
                                tile_position=(h * 64, 0))
                    nc.tensor.ldweights(w_sb[:, 0:128])
                    for h in range(2):
                        pt = ptp.tile([128, 1024], BF, tag="pt", name="pt")
                        nc.scalar.activation(pt[:], pss[h][:], EXP)
                        for u in range(2):
                            jb = b * 16 + 2 * cc + u
                            nc.tensor.matmul(
                                pv[h][:],
                                v_sb[:, 130 * jb + 65 * h:
                                     130 * jb + 65 * h + 65],
                                pt[:, u * 512:(u + 1) * 512],
                                start=(cc == 0 and u == 0), stop=False)
                # diagonal j-blocks: shrink to valid columns, batch rr
                # pairs into one psum tile / one exp, triangle mask
                for rp in range(2):            # rr pair: (0,1) or (2,3)
                    rrs = (2 * rp, 2 * rp + 1)
                    ws = [512 - 128 * rr for rr in rrs]
                    pss = [psS.tile([128, 1024], F32, tag="s",
                                    name=f"psd{hh}") for hh in range(2)]
                    for ui, rr in enumerate(rrs):
                        off = 128 * rr
                        c0 = 0 if ui == 0 else ws[0]
                        for h in range(2):
                            jb = b * 16 + 4 * I + rr
                            nc.tensor.matmul(
                                pss[h][:, c0: c0 + ws[ui]],
                                kT[h * 64:(h + 1) * 64,
                                   jb * 128: jb * 128 + 128],
                                qT[h * 64:(h + 1) * 64,
                                   icol + off: icol + 512],
                                start=True, stop=True,
                                tile_position=(h * 64, 0))
                    nc.tensor.ldweights(w_sb[:, 0:128])
                    for h in range(2):
                        pt = ptp.tile([128, 1024], BF, tag="pt", name="ptd")
                        wtot = ws[0] + ws[1]
                        nc.scalar.activation(pt[:, 0:wtot],
                                             pss[h][:, 0:wtot], EXP)
                        for ui, rr in enumerate(rrs):
                            c0 = 0 if ui == 0 else ws[0]
                            nc.vector.tensor_mul(
                                pt[:, c0: c0 + 128], pt[:, c0: c0 + 128],
                                mask_sb[:, 384:512])
                            jb = b * 16 + 4 * I + rr
                            nc.tensor.matmul(
                                pv[h][:, 128 * rr: 512],
                                v_sb[:, 130 * jb + 65 * h:
                                     130 * jb + 65 * h + 65],
                                pt[:, c0: c0 + ws[ui]],
                                start=(I == 0 and rr == 0), stop=(rr == 3))
                # normalize out^T[d, i] by softmax denominator (pv row 64)
                for h in range(2):
                    pvb = smp.tile([65, 512], F32, tag="pvb", name="pvb")
                    nc.vector.tensor_copy(pvb[:], pv[h][:])
                    lr = smp.tile([1, 512], F32, tag="lr", name="lr")
                    nc.vector.tensor_copy(lr[:], pvb[64:65, :])
                    ell = smp.tile([1, 512], F32, tag="ell", name="ell")
                    nc.vector.reciprocal_approx_fast(ell[:], lr[:])
                    ellb = smp.tile([1, 512], BF, tag="ellb", name="ellb")
                    nc.vector.tensor_copy(ellb[:], ell[:])
                    pb = psQ.tile([64, 512], F32, tag="q", name="pb")
                    nc.tensor.matmul(pb[:], ones_sb[0:1, 0:64], ellb[0:1, :],
                                     start=True, stop=True)
                    rb = smp.tile([64, 512], BF, tag="rb", name="rb")
                    nc.vector.tensor_copy(rb[:], pb[:])
                    nc.vector.tensor_mul(
                        staged[h * 64:(h + 1) * 64, icol:icol + 512],
                        pvb[0:64, :], rb[:])

            # ---- exchange waves + projection ------------------------
            def exchange(wave, src0, width, ain, aout):
                # chunk j = staged[:, src0 + width*j : +width]
                nc.gpsimd.dma_start(
                    ain[:].rearrange("(c p) i -> p c i", p=128),
                    staged[:, src0: src0 + NCORES * width]
                    .rearrange("p (c i) -> p c i", c=NCORES))
                nc.gpsimd.collective_compute(
                    "AllToAll", mybir.AluOpType.bypass,
                    replica_groups=[list(range(NCORES))],
                    ins=[ain[:]], outs=[aout[:]])

            def load_po(po, aout, width):
                nc.sync.dma_start(
                    po[:].rearrange("p (c i) -> p c i", c=NCORES),
                    aout[:].rearrange("(c p) i -> p c i", p=128))

            def proj_tb(tb, po, width):
                # y rows tb*128.. from po (lhsT: [dims, 128 tokens])
                ci = (tb * 128) % width
                for co in range(2):
                    ps = psQ.tile([128, 512], F32, tag="q", name="psy")
                    for kk in range(NKT):
                        nc.tensor.matmul(
                            ps[:],
                            po[:, kk * width + ci: kk * width + ci + 128],
                            wp_sb[:, kk * C + co * 512:
                                  kk * C + co * 512 + 512],
                            start=(kk == 0), stop=(kk == NKT - 1))
                    ysb = yp.tile([128, 512], F32, tag="y", name="ysb")
                    nc.vector.tensor_add(
                        ysb[:], ps[:], bias_sb[:, co * 512:co * 512 + 512])
                    nc.sync.dma_start(
                        y_d[tb * 128:(tb + 1) * 128,
                            co * 512:(co + 1) * 512], ysb[:])

            y2st = cst.tile([128, 4 * C], F32, name="y2st")

            def pproj(k):
                # tokens 3072+512k .. +512 of staged -> y2 rows 512k..
                for tb in range(4):
                    g0 = 3072 + 512 * k + 128 * tb
                    for co in range(2):
                        ps = psQ.tile([128, 512], F32, tag="q", name="psp")
                        nc.tensor.matmul(
                            ps[:], staged[:, g0: g0 + 128],
                            wpo_sb[:, co * 512: co * 512 + 512],
                            start=True, stop=True)
                        nc.vector.tensor_copy(
                            y2st[:, tb * C + co * 512:
                                 tb * C + co * 512 + 512], ps[:])
                nc.sync.dma_start(
                    y2_d[512 * k: 512 * k + 512, :]
                    .rearrange("(tb p) n -> p tb n", p=128),
                    y2st[:].rearrange("p (tb n) -> p tb n", tb=4))

            for I in range(4):          # b0: qkv one step ahead of attn
                qkv_block(0, I, I + 1)
                if I < 3:
                    attn_block(0, I)
            attn_block(0, 3)
            qkv_block(1, 0, 1)
            attn_block(1, 0)
            qkv_block(1, 1, 2)
            exchange(0, 0, HTS, a2a_in[0], a2a_out[0])       # b0, hidden
            qkv_block(1, 2, 3)
            attn_block(1, 1)
            qkv_block(1, 3, 4)
            exchange(1, T, 128, a2a_in[1], a2a_out[1])       # b1 1st half
            load_po(po0, a2a_out[0], HTS)
            attn_block(1, 2)
            proj_tb(0, po0, HTS)
            proj_tb(1, po0, HTS)
            pproj(0)               # tokens 1024-1535, local partial
            load_po(po1a, a2a_out[1], 128)
            proj_tb(2, po1a, 128)
            attn_block(1, 3)
            pproj(1)               # tokens 1536-2047, local partial


    nc.compile()
    return nc


def _prep_inputs(x, w_qkv, w_proj, b_proj):
    bf = ml_dtypes.bfloat16
    xT = np.ascontiguousarray(x.reshape(G, C).T).astype(bf)
    wpT = np.ascontiguousarray(w_proj.T).astype(bf)
    bias = np.ascontiguousarray(
        np.broadcast_to(b_proj.astype(np.float32), (128, C)))
    mask = (np.arange(896)[None, :] - 384 >=
            np.arange(128)[:, None]).astype(bf)
    scale = np.float32(HS ** -0.5)
    in_maps = []
    for c in range(NCORES):
        h0, h1 = 2 * c, 2 * c + 1
        cols = []
        for part, sc in ((slice(0, 64), scale), (slice(64, 128), None),
                         (slice(128, 192), None)):
            for h in (h0, h1):
                w = w_qkv[h, part, :]
                if sc is not None:
                    w = w * sc
                cols.append(np.ascontiguousarray(w.T))
        wc = np.concatenate(cols, axis=1).astype(bf)   # [C, 384]
        wpo = np.ascontiguousarray(wpT[128 * c:128 * (c + 1), :]).astype(bf)
        in_maps.append({"xT": xT, "wqkv": wc, "wpT": wpT,
                        "bias": bias, "mask": mask, "wpo": wpo})
    return in_maps


def _get_nc():
    global _CACHED
    if _CACHED is None:
        _CACHED = _build()
    return _CACHED


def run_on_cores(in_maps, **kwargs):
    nc = _get_nc()
    return bass_utils.run_bass_kernel_spmd(
        nc, in_maps, core_ids=list(range(NCORES)), **kwargs)


def kernel(x, w_qkv, w_proj, b_proj):
    in_maps = _prep_inputs(x, w_qkv, w_proj, b_proj)
    res = run_on_cores(in_maps)
    y = np.empty((B, T, C), dtype=np.float32)
    acc = None
    for c in range(NCORES):
        yc = res.results[c]["y"]
        y[0, HTS * c: HTS * (c + 1), :] = yc[0:256]
        y[1, 128 * c: 128 * (c + 1), :] = yc[256:384]
        y2 = res.results[c]["y2"]
        acc = y2 if acc is None else acc + y2
    y[1, 1024:2048, :] = acc + b_proj.astype(np.float32)[None, :]
    return y


# revision 16
# speedup vs baseline: 1.2432x; 1.0405x over previous
"""Multihead causal attention block on 8 Trainium2 NeuronCores.

Sharding: tensor-parallel over heads (2 heads/core). Each core computes
qkv + attention for its heads over all tokens; two AllToAlls (one per
batch element, pipelined against attention compute) redistribute
attention outputs so each core holds all 1024 feature dims for two
256-token half-slices, where it runs the output projection locally.

Fixed problem shape: B=2, T=2048, C=1024, H=16, HS=64.
"""

import sys

sys.path.insert(0, "/opt/trn_rl_repo")

import numpy as np
import ml_dtypes

import concourse.bass as bass
import concourse.tile as tile
from concourse import bacc, mybir
from concourse import bass_utils

B, T, C = 2, 2048, 1024
H, HS = 16, 64
G = B * T              # 4096 global tokens (b-major)
NCORES = 8
NKT = C // 128         # 8 contraction tiles
HTS = T // NCORES      # 256-token half-slice per core per batch

dt = mybir.dt
BF = dt.bfloat16
F32 = dt.float32
EXP = mybir.ActivationFunctionType.Exp

_CACHED = None


def _build():
    nc = bacc.Bacc("TRN2", target_bir_lowering=False, debug=False,
                   num_devices=NCORES)

    xT_d = nc.dram_tensor("xT", [C, G], BF, kind="ExternalInput")
    wqkv_d = nc.dram_tensor("wqkv", [C, 384], BF, kind="ExternalInput")
    wpT_d = nc.dram_tensor("wpT", [C, C], BF, kind="ExternalInput")
    bias_d = nc.dram_tensor("bias", [128, C], F32, kind="ExternalInput")
    mask_d = nc.dram_tensor("mask", [128, 896], BF, kind="ExternalInput")
    y_d = nc.dram_tensor("y", [384, C], F32, kind="ExternalOutput")
    y2_d = nc.dram_tensor("y2", [1024, C], F32, kind="ExternalOutput")
    wpo_d = nc.dram_tensor("wpo", [128, C], BF, kind="ExternalInput")

    a2a_shapes = [HTS, 128]
    a2a_in = [nc.dram_tensor(f"a2a_in{w}", [NCORES * 128, a2a_shapes[w]], BF)
              for w in range(2)]
    a2a_out = [nc.dram_tensor(f"a2a_out{w}", [NCORES * 128, a2a_shapes[w]], BF)
               for w in range(2)]

    with tile.TileContext(nc) as tc:
        with tc.tile_pool(name="cst", bufs=1) as cst, \
             tc.tile_pool(name="pt", bufs=3) as ptp, \
             tc.tile_pool(name="sm", bufs=4) as smp, \
             tc.tile_pool(name="yp", bufs=3) as yp, \
             tc.tile_pool(name="psS", bufs=2, space="PSUM") as psS, \
             tc.tile_pool(name="psPV", bufs=2, space="PSUM") as psPV, \
             tc.tile_pool(name="psQ", bufs=2, space="PSUM") as psQ:

            # ---- constant loads -------------------------------------
            x_sb = cst.tile([128, NKT * G], BF)        # x^T c-tiles
            w_sb = cst.tile([128, NKT * 384], BF)      # per-head qkv weights
            mask_sb = cst.tile([128, 896], BF)
            bias_sb = cst.tile([128, C], F32)
            wp_sb = cst.tile([128, NKT * C], BF)       # w_proj^T c-tiles
            qT = cst.tile([128, G], BF)                # q^T (2 heads stacked)
            kT = cst.tile([128, G], BF)
            v_sb = cst.tile([128, 32 * 130], BF)       # v natural + ones col
            staged = cst.tile([128, G], BF)            # normalized out^T
            ones_sb = cst.tile([1, 64], BF)
            po0 = cst.tile([128, NCORES * 256], BF)    # b0 tokens 256j
            po1a = cst.tile([128, NCORES * 128], BF)   # b1 tokens 128j
            wpo_sb = cst.tile([128, C], BF)            # own rows of w_proj^T

            nc.sync.dma_start(
                w_sb[:].rearrange("p (k n) -> p k n", k=NKT),
                wqkv_d[:].rearrange("(k p) n -> p k n", p=128))
            # x in priority waves of single wide DMAs (HWDGE fans each
            # across all 16 SDMA engines); earliest tokens first
            x_sb_v = x_sb[:].rearrange("p (k g) -> p k g", k=NKT)
            xT_v = xT_d[:].rearrange("(k p) g -> p k g", p=128)
            for w0, w1 in ((0, 512), (512, 1024), (1024, T),
                           (T, 3 * 1024), (3 * 1024, G)):
                nc.sync.dma_start(x_sb_v[:, :, w0:w1], xT_v[:, :, w0:w1])
            nc.sync.dma_start(mask_sb[:], mask_d[:])
            nc.sync.dma_start(wpo_sb[:], wpo_d[:])
            nc.sync.dma_start(bias_sb[:], bias_d[:])
            nc.sync.dma_start(
                wp_sb[:].rearrange("p (k n) -> p k n", k=NKT),
                wpT_d[:].rearrange("(k p) n -> p k n", p=128))

            nc.vector.memset(ones_sb[:], 1.0)
            # ones column of v_aug: offsets 64 + 65*m
            v_ones = v_sb[:].rearrange("p (m o) -> p m o", o=65)[:, :, 64:65]
            nc.vector.memset(v_ones, 1.0)

            # ---- qkv projections ------------------------------------
            def qkv_block(b, tb0=0, tb1=4):
                for tb in range(tb0, tb1):
                    gt = b * T + tb * 512
                    for part in range(2):      # 0=q pair, 1=k pair
                        ps = psQ.tile([128, 512], F32, tag="q", name="psqk")
                        for kk in range(NKT):
                            nc.tensor.matmul(
                                ps[:],
                                w_sb[:, kk * 384 + part * 128:
                                     kk * 384 + part * 128 + 128],
                                x_sb[:, kk * G + gt: kk * G + gt + 512],
                                start=(kk == 0), stop=(kk == NKT - 1))
                        dst = qT if part == 0 else kT
                        nc.vector.tensor_copy(dst[:, gt:gt + 512], ps[:])
                    for ts in range(4):        # v in natural layout
                        g0 = gt + 128 * ts
                        jb = g0 // 128
                        ps = psQ.tile([128, 128], F32, tag="q", name="psv")
                        for kk in range(NKT):
                            nc.tensor.matmul(
                                ps[:],
                                x_sb[:, kk * G + g0: kk * G + g0 + 128],
                                w_sb[:, kk * 384 + 256: kk * 384 + 384],
                                start=(kk == 0), stop=(kk == NKT - 1))
                        dst = v_sb[:, 130 * jb: 130 * jb + 130] \
                            .rearrange("p (h o) -> p h o", o=65)[:, :, 0:64]
                        nc.vector.tensor_copy(
                            dst, ps[:].rearrange("p (h d) -> p h d", d=64))

            # ---- attention for one (b, I) i-block of 512 ------------
            def attn_block(b, I):
                icol = (b * 4 + I) * 512
                pv = [psPV.tile([65, 512], F32, tag="pv", name=f"pv{b}{I}{hh}")
                      for hh in range(2)]
                # off-diagonal j-blocks, chunks of 2 (no mask needed)
                for cc in range(2 * I):
                    pss = [psS.tile([128, 1024], F32, tag="s",
                                    name=f"pss{hh}") for hh in range(2)]
                    for u in range(2):
                        for h in range(2):
                            jb = b * 16 + 2 * cc + u
                            nc.tensor.matmul(
                                pss[h][:, u * 512:(u + 1) * 512],
                                kT[h * 64:(h + 1) * 64,
                                   jb * 128: jb * 128 + 128],
                                qT[h * 64:(h + 1) * 64, icol: icol + 512],
                                start=True, stop=True,
                                tile_position=(h * 64, 0))
                    for h in range(2):
                        pt = ptp.tile([128, 1024], BF, tag="pt", name="pt")
                        nc.scalar.activation(pt[:], pss[h][:], EXP)
                        for u in range(2):
                            jb = b * 16 + 2 * cc + u
                            nc.tensor.matmul(
                                pv[h][:],
                                v_sb[:, 130 * jb + 65 * h:
                                     130 * jb + 65 * h + 65],
                                pt[:, u * 512:(u + 1) * 512],
                                start=(cc == 0 and u == 0), stop=False)
                # diagonal j-blocks: shrink to valid columns, batch rr
                # pairs into one psum tile / one exp, triangle mask
                for rp in range(2):            # rr pair: (0,1) or (2,3)
                    rrs = (2 * rp, 2 * rp + 1)
                    ws = [512 - 128 * rr for rr in rrs]
                    pss = [psS.tile([128, 1024], F32, tag="s",
                                    name=f"psd{hh}") for hh in range(2)]
                    for ui, rr in enumerate(rrs):
                        off = 128 * rr
                        c0 = 0 if ui == 0 else ws[0]
                        for h in range(2):
                            jb = b * 16 + 4 * I + rr
                            nc.tensor.matmul(
                                pss[h][:, c0: c0 + ws[ui]],
                                kT[h * 64:(h + 1) * 64,
                                   jb * 128: jb * 128 + 128],
                                qT[h * 64:(h + 1) * 64,
                                   icol + off: icol + 512],
                                start=True, stop=True,
                                tile_position=(h * 64, 0))
                    for h in range(2):
                        pt = ptp.tile([128, 1024], BF, tag="pt", name="ptd")
                        wtot = ws[0] + ws[1]
                        nc.scalar.activation(pt[:, 0:wtot],
                                             pss[h][:, 0:wtot], EXP)
                        for ui, rr in enumerate(rrs):
                            c0 = 0 if ui == 0 else ws[0]
                            nc.vector.tensor_mul(
                                pt[:, c0: c0 + 128], pt[:, c0: c0 + 128],
                                mask_sb[:, 384:512])
                            jb = b * 16 + 4 * I + rr
                            nc.tensor.matmul(
                                pv[h][:, 128 * rr: 512],
                                v_sb[:, 130 * jb + 65 * h:
                                     130 * jb + 65 * h + 65],
                                pt[:, c0: c0 + ws[ui]],
                                start=(I == 0 and rr == 0), stop=(rr == 3))
                # normalize out^T[d, i] by softmax denominator (pv row 64)
                for h in range(2):
                    pvb = smp.tile([65, 512], F32, tag="pvb", name="pvb")
                    nc.vector.tensor_copy(pvb[:], pv[h][:])
                    lr = smp.tile([1, 512], F32, tag="lr", name="lr")
                    nc.vector.tensor_copy(lr[:], pvb[64:65, :])
                    ell = smp.tile([1, 512], F32, tag="ell", name="ell")
                    nc.vector.reciprocal_approx_fast(ell[:], lr[:])
                    ellb = smp.tile([1, 512], BF, tag="ellb", name="ellb")
                    nc.vector.tensor_copy(ellb[:], ell[:])
                    pb = psQ.tile([64, 512], F32, tag="q", name="pb")
                    nc.tensor.matmul(pb[:], ones_sb[0:1, 0:64], ellb[0:1, :],
                                     start=True, stop=True)
                    rb = smp.tile([64, 512], BF, tag="rb", name="rb")
                    nc.vector.tensor_copy(rb[:], pb[:])
                    nc.vector.tensor_mul(
                        staged[h * 64:(h + 1) * 64, icol:icol + 512],
                        pvb[0:64, :], rb[:])

            # ---- exchange waves + projection ------------------------
            def exchange(wave, src0, width, ain, aout):
                # chunk j = staged[:, src0 + width*j : +width]
                nc.gpsimd.dma_start(
                    ain[:].rearrange("(c p) i -> p c i", p=128),
                    staged[:, src0: src0 + NCORES * width]
                    .rearrange("p (c i) -> p c i", c=NCORES))
                nc.gpsimd.collective_compute(
                    "AllToAll", mybir.AluOpType.bypass,
                    replica_groups=[list(range(NCORES))],
                    ins=[ain[:]], outs=[aout[:]])

            def load_po(po, aout, width):
                nc.sync.dma_start(
                    po[:].rearrange("p (c i) -> p c i", c=NCORES),
                    aout[:].rearrange("(c p) i -> p c i", p=128))

            def proj_tb(tb, po, width):
                # y rows tb*128.. from po (lhsT: [dims, 128 tokens])
                ci = (tb * 128) % width
                for co in range(2):
                    ps = psQ.tile([128, 512], F32, tag="q", name="psy")
                    for kk in range(NKT):
                        nc.tensor.matmul(
                            ps[:],
                            po[:, kk * width + ci: kk * width + ci + 128],
                            wp_sb[:, kk * C + co * 512:
                                  kk * C + co * 512 + 512],
                            start=(kk == 0), stop=(kk == NKT - 1))
                    ysb = yp.tile([128, 512], F32, tag="y", name="ysb")
                    nc.vector.tensor_add(
                        ysb[:], ps[:], bias_sb[:, co * 512:co * 512 + 512])
                    nc.sync.dma_start(
                        y_d[tb * 128:(tb + 1) * 128,
                            co * 512:(co + 1) * 512], ysb[:])

            y2st = cst.tile([128, 4 * C], F32, name="y2st")

            def pproj(k):
                # tokens 3072+512k .. +512 of staged -> y2 rows 512k..
                for tb in range(4):
                    g0 = 3072 + 512 * k + 128 * tb
                    for co in range(2):
                        ps = psQ.tile([128, 512], F32, tag="q", name="psp")
                        nc.tensor.matmul(
                            ps[:], staged[:, g0: g0 + 128],
                            wpo_sb[:, co * 512: co * 512 + 512],
                            start=True, stop=True)
                        nc.vector.tensor_copy(
                            y2st[:, tb * C + co * 512:
                                 tb * C + co * 512 + 512], ps[:])
                nc.sync.dma_start(
                    y2_d[512 * k: 512 * k + 512, :]
                    .rearrange("(tb p) n -> p tb n", p=128),
                    y2st[:].rearrange("p (tb n) -> p tb n", tb=4))

            for I in range(4):          # b0: qkv one step ahead of attn
                qkv_block(0, I, I + 1)
                if I < 3:
                    attn_block(0, I)
            attn_block(0, 3)
            qkv_block(1, 0, 1)
            attn_block(1, 0)
            qkv_block(1, 1, 2)
            exchange(0, 0, HTS, a2a_in[0], a2a_out[0])       # b0, hidden
            qkv_block(1, 2, 3)
            attn_block(1, 1)
            qkv_block(1, 3, 4)
            exchange(1, T, 128, a2a_in[1], a2a_out[1])       # b1 1st half
            load_po(po0, a2a_out[0], HTS)
            attn_block(1, 2)
            proj_tb(0, po0, HTS)
            proj_tb(1, po0, HTS)
            pproj(0)               # tokens 1024-1535, local partial
            load_po(po1a, a2a_out[1], 128)
            attn_block(1, 3)
            pproj(1)               # tokens 1536-2047, local partial
            proj_tb(2, po1a, 128)


    nc.compile()
    return nc


def _prep_inputs(x, w_qkv, w_proj, b_proj):
    bf = ml_dtypes.bfloat16
    xT = np.ascontiguousarray(x.reshape(G, C).T).astype(bf)
    wpT = np.ascontiguousarray(w_proj.T).astype(bf)
    bias = np.ascontiguousarray(
        np.broadcast_to(b_proj.astype(np.float32), (128, C)))
    mask = (np.arange(896)[None, :] - 384 >=
            np.arange(128)[:, None]).astype(bf)
    scale = np.float32(HS ** -0.5)
    in_maps = []
    for c in range(NCORES):
        h0, h1 = 2 * c, 2 * c + 1
        cols = []
        for part, sc in ((slice(0, 64), scale), (slice(64, 128), None),
                         (slice(128, 192), None)):
            for h in (h0, h1):
                w = w_qkv[h, part, :]
                if sc is not None:
                    w = w * sc
                cols.append(np.ascontiguousarray(w.T))
        wc = np.concatenate(cols, axis=1).astype(bf)   # [C, 384]
        wpo = np.ascontiguousarray(wpT[128 * c:128 * (c + 1), :]).astype(bf)
        in_maps.append({"xT": xT, "wqkv": wc, "wpT": wpT,
                        "bias": bias, "mask": mask, "wpo": wpo})
    return in_maps


def _get_nc():
    global _CACHED
    if _CACHED is None:
        _CACHED = _build()
    return _CACHED


def run_on_cores(in_maps, **kwargs):
    nc = _get_nc()
    return bass_utils.run_bass_kernel_spmd(
        nc, in_maps, core_ids=list(range(NCORES)), **kwargs)


def kernel(x, w_qkv, w_proj, b_proj):
    in_maps = _prep_inputs(x, w_qkv, w_proj, b_proj)
    res = run_on_cores(in_maps)
    y = np.empty((B, T, C), dtype=np.float32)
    acc = None
    for c in range(NCORES):
        yc = res.results[c]["y"]
        y[0, HTS * c: HTS * (c + 1), :] = yc[0:256]
        y[1, 128 * c: 128 * (c + 1), :] = yc[256:384]
        y2 = res.results[c]["y2"]
        acc = y2 if acc is None else acc + y2
    y[1, 1024:2048, :] = acc + b_proj.astype(np.float32)[None, :]
    return y


# revision 17
# speedup vs baseline: 1.2442x; 1.0008x over previous
"""Multihead causal attention block on 8 Trainium2 NeuronCores.

Sharding: tensor-parallel over heads (2 heads/core). Each core computes
qkv + attention for its heads over all tokens; two AllToAlls (one per
batch element, pipelined against attention compute) redistribute
attention outputs so each core holds all 1024 feature dims for two
256-token half-slices, where it runs the output projection locally.

Fixed problem shape: B=2, T=2048, C=1024, H=16, HS=64.
"""

import sys

sys.path.insert(0, "/opt/trn_rl_repo")

import numpy as np
import ml_dtypes

import concourse.bass as bass
import concourse.tile as tile
from concourse import bacc, mybir
from concourse import bass_utils

B, T, C = 2, 2048, 1024
H, HS = 16, 64
G = B * T              # 4096 global tokens (b-major)
NCORES = 8
NKT = C // 128         # 8 contraction tiles
HTS = T // NCORES      # 256-token half-slice per core per batch

dt = mybir.dt
BF = dt.bfloat16
F32 = dt.float32
EXP = mybir.ActivationFunctionType.Exp

_CACHED = None


def _build():
    nc = bacc.Bacc("TRN2", target_bir_lowering=False, debug=False,
                   num_devices=NCORES)

    xT_d = nc.dram_tensor("xT", [C, G], BF, kind="ExternalInput")
    wqkv_d = nc.dram_tensor("wqkv", [C, 384], BF, kind="ExternalInput")
    wpT_d = nc.dram_tensor("wpT", [C, C], BF, kind="ExternalInput")
    bias_d = nc.dram_tensor("bias", [128, C], F32, kind="ExternalInput")
    mask_d = nc.dram_tensor("mask", [128, 896], BF, kind="ExternalInput")
    y_d = nc.dram_tensor("y", [384, C], F32, kind="ExternalOutput")
    y2_d = nc.dram_tensor("y2", [1024, C], F32, kind="ExternalOutput")
    wpo_d = nc.dram_tensor("wpo", [128, C], BF, kind="ExternalInput")

    a2a_shapes = [HTS, 128]
    a2a_in = [nc.dram_tensor(f"a2a_in{w}", [NCORES * 128, a2a_shapes[w]], BF)
              for w in range(2)]
    a2a_out = [nc.dram_tensor(f"a2a_out{w}", [NCORES * 128, a2a_shapes[w]], BF)
               for w in range(2)]

    with tile.TileContext(nc) as tc:
        with tc.tile_pool(name="cst", bufs=1) as cst, \
             tc.tile_pool(name="pt", bufs=3) as ptp, \
             tc.tile_pool(name="sm", bufs=4) as smp, \
             tc.tile_pool(name="yp", bufs=3) as yp, \
             tc.tile_pool(name="psS", bufs=2, space="PSUM") as psS, \
             tc.tile_pool(name="psPV", bufs=2, space="PSUM") as psPV, \
             tc.tile_pool(name="psQ", bufs=2, space="PSUM") as psQ:

            # ---- constant loads -------------------------------------
            x_sb = cst.tile([128, NKT * G], BF)        # x^T c-tiles
            w_sb = cst.tile([128, NKT * 384], BF)      # per-head qkv weights
            mask_sb = cst.tile([128, 896], BF)
            bias_sb = cst.tile([128, C], F32)
            wp_sb = cst.tile([128, NKT * C], BF)       # w_proj^T c-tiles
            qT = cst.tile([128, G], BF)                # q^T (2 heads stacked)
            kT = cst.tile([128, G], BF)
            v_sb = cst.tile([128, 32 * 130], BF)       # v natural + ones col
            staged = cst.tile([128, G], BF)            # normalized out^T
            ones_sb = cst.tile([1, 64], BF)
            po0 = cst.tile([128, NCORES * 256], BF)    # b0 tokens 256j
            po1a = cst.tile([128, NCORES * 128], BF)   # b1 tokens 128j
            wpo_sb = cst.tile([128, C], BF)            # own rows of w_proj^T

            nc.sync.dma_start(
                w_sb[:].rearrange("p (k n) -> p k n", k=NKT),
                wqkv_d[:].rearrange("(k p) n -> p k n", p=128))
            # x in priority waves of single wide DMAs (HWDGE fans each
            # across all 16 SDMA engines); earliest tokens first
            x_sb_v = x_sb[:].rearrange("p (k g) -> p k g", k=NKT)
            xT_v = xT_d[:].rearrange("(k p) g -> p k g", p=128)
            for w0, w1 in ((0, 512), (512, 1024), (1024, T),
                           (T, 3 * 1024), (3 * 1024, G)):
                nc.sync.dma_start(x_sb_v[:, :, w0:w1], xT_v[:, :, w0:w1])
            nc.sync.dma_start(mask_sb[:], mask_d[:])
            nc.sync.dma_start(wpo_sb[:], wpo_d[:])
            nc.sync.dma_start(bias_sb[:], bias_d[:])
            nc.sync.dma_start(
                wp_sb[:].rearrange("p (k n) -> p k n", k=NKT),
                wpT_d[:].rearrange("(k p) n -> p k n", p=128))

            nc.vector.memset(ones_sb[:], 1.0)
            # ones column of v_aug: offsets 64 + 65*m
            v_ones = v_sb[:].rearrange("p (m o) -> p m o", o=65)[:, :, 64:65]
            nc.vector.memset(v_ones, 1.0)

            # ---- qkv projections ------------------------------------
            def qkv_block(b, tb0=0, tb1=4):
                for tb in range(tb0, tb1):
                    gt = b * T + tb * 512
                    for part in range(2):      # 0=q pair, 1=k pair
                        ps = psQ.tile([128, 512], F32, tag="q", name="psqk")
                        for kk in range(NKT):
                            nc.tensor.matmul(
                                ps[:],
                                w_sb[:, kk * 384 + part * 128:
                                     kk * 384 + part * 128 + 128],
                                x_sb[:, kk * G + gt: kk * G + gt + 512],
                                start=(kk == 0), stop=(kk == NKT - 1))
                        dst = qT if part == 0 else kT
                        nc.vector.tensor_copy(dst[:, gt:gt + 512], ps[:])
                    for ts in range(4):        # v in natural layout
                        g0 = gt + 128 * ts
                        jb = g0 // 128
                        ps = psQ.tile([128, 128], F32, tag="q", name="psv")
                        for kk in range(NKT):
                            nc.tensor.matmul(
                                ps[:],
                                x_sb[:, kk * G + g0: kk * G + g0 + 128],
                                w_sb[:, kk * 384 + 256: kk * 384 + 384],
                                start=(kk == 0), stop=(kk == NKT - 1))
                        dst = v_sb[:, 130 * jb: 130 * jb + 130] \
                            .rearrange("p (h o) -> p h o", o=65)[:, :, 0:64]
                        nc.vector.tensor_copy(
                            dst, ps[:].rearrange("p (h d) -> p h d", d=64))

            # ---- attention for one (b, I) i-block of 512 ------------
            def attn_block(b, I):
                icol = (b * 4 + I) * 512
                pv = [psPV.tile([65, 512], F32, tag="pv", name=f"pv{b}{I}{hh}")
                      for hh in range(2)]
                # off-diagonal j-blocks, chunks of 2 (no mask needed)
                for cc in range(2 * I):
                    pss = [psS.tile([128, 1024], F32, tag="s",
                                    name=f"pss{hh}") for hh in range(2)]
                    for u in range(2):
                        for h in range(2):
                            jb = b * 16 + 2 * cc + u
                            nc.tensor.matmul(
                                pss[h][:, u * 512:(u + 1) * 512],
                                kT[h * 64:(h + 1) * 64,
                                   jb * 128: jb * 128 + 128],
                                qT[h * 64:(h + 1) * 64, icol: icol + 512],
                                start=True, stop=True,
                                tile_position=(h * 64, 0))
                    for h in range(2):
                        pt = ptp.tile([128, 1024], BF, tag="pt", name="pt")
                        nc.scalar.activation(pt[:], pss[h][:], EXP)
                        for u in range(2):
                            jb = b * 16 + 2 * cc + u
                            nc.tensor.matmul(
                                pv[h][:],
                                v_sb[:, 130 * jb + 65 * h:
                                     130 * jb + 65 * h + 65],
                                pt[:, u * 512:(u + 1) * 512],
                                start=(cc == 0 and u == 0), stop=False)
                # diagonal j-blocks: shrink to valid columns, batch rr
                # pairs into one psum tile / one exp, triangle mask
                for rp in range(2):            # rr pair: (0,1) or (2,3)
                    rrs = (2 * rp, 2 * rp + 1)
                    ws = [512 - 128 * rr for rr in rrs]
                    pss = [psS.tile([128, 1024], F32, tag="s",
                                    name=f"psd{hh}") for hh in range(2)]
                    for ui, rr in enumerate(rrs):
                        off = 128 * rr
                        c0 = 0 if ui == 0 else ws[0]
                        for h in range(2):
                            jb = b * 16 + 4 * I + rr
                            nc.tensor.matmul(
                                pss[h][:, c0: c0 + ws[ui]],
                                kT[h * 64:(h + 1) * 64,
                                   jb * 128: jb * 128 + 128],
                                qT[h * 64:(h + 1) * 64,
                                   icol + off: icol + 512],
                                start=True, stop=True,
                                tile_position=(h * 64, 0))
                    for h in range(2):
                        pt = ptp.tile([128, 1024], BF, tag="pt", name="ptd")
                        wtot = ws[0] + ws[1]
                        nc.scalar.activation(pt[:, 0:wtot],
                                             pss[h][:, 0:wtot], EXP)
                        for ui, rr in enumerate(rrs):
                            c0 = 0 if ui == 0 else ws[0]
                            nc.vector.tensor_mul(
                                pt[:, c0: c0 + 128], pt[:, c0: c0 + 128],
                                mask_sb[:, 384:512])
                            jb = b * 16 + 4 * I + rr
                            nc.tensor.matmul(
                                pv[h][:, 128 * rr: 512],
                                v_sb[:, 130 * jb + 65 * h:
                                     130 * jb + 65 * h + 65],
                                pt[:, c0: c0 + ws[ui]],
                                start=(I == 0 and rr == 0), stop=(rr == 3))
                # normalize, part 1 (immediate): evacuate PSUM + recip;
                # part 2 (returned closure, emitted one block later so the
                # PE never stalls on the DVE recip chain): bcast + scale
                fins = []
                for h in range(2):
                    pvb = smp.tile([65, 512], F32, tag="pvb", name="pvb")
                    nc.vector.tensor_copy(pvb[:], pv[h][:])
                    lr = smp.tile([1, 512], F32, tag="lr", name="lr")
                    nc.vector.tensor_copy(lr[:], pvb[64:65, :])
                    ell = smp.tile([1, 512], F32, tag="ell", name="ell")
                    nc.vector.reciprocal_approx_fast(ell[:], lr[:])
                    ellb = smp.tile([1, 512], BF, tag="ellb", name="ellb")
                    nc.vector.tensor_copy(ellb[:], ell[:])
                    fins.append((pvb, ellb))

                def fin(icol=icol, fins=fins):
                    for h, (pvb, ellb) in enumerate(fins):
                        pb = psQ.tile([64, 512], F32, tag="q", name="pb")
                        nc.tensor.matmul(pb[:], ones_sb[0:1, 0:64],
                                         ellb[0:1, :], start=True, stop=True)
                        rb = smp.tile([64, 512], BF, tag="rb", name="rb")
                        nc.vector.tensor_copy(rb[:], pb[:])
                        nc.vector.tensor_mul(
                            staged[h * 64:(h + 1) * 64, icol:icol + 512],
                            pvb[0:64, :], rb[:])
                return fin

            # ---- exchange waves + projection ------------------------
            def exchange(wave, src0, width, ain, aout):
                # chunk j = staged[:, src0 + width*j : +width]
                nc.gpsimd.dma_start(
                    ain[:].rearrange("(c p) i -> p c i", p=128),
                    staged[:, src0: src0 + NCORES * width]
                    .rearrange("p (c i) -> p c i", c=NCORES))
                nc.gpsimd.collective_compute(
                    "AllToAll", mybir.AluOpType.bypass,
                    replica_groups=[list(range(NCORES))],
                    ins=[ain[:]], outs=[aout[:]])

            def load_po(po, aout, width):
                nc.sync.dma_start(
                    po[:].rearrange("p (c i) -> p c i", c=NCORES),
                    aout[:].rearrange("(c p) i -> p c i", p=128))

            def proj_tb(tb, po, width):
                # y rows tb*128.. from po (lhsT: [dims, 128 tokens])
                ci = (tb * 128) % width
                for co in range(2):
                    ps = psQ.tile([128, 512], F32, tag="q", name="psy")
                    for kk in range(NKT):
                        nc.tensor.matmul(
                            ps[:],
                            po[:, kk * width + ci: kk * width + ci + 128],
                            wp_sb[:, kk * C + co * 512:
                                  kk * C + co * 512 + 512],
                            start=(kk == 0), stop=(kk == NKT - 1))
                    ysb = yp.tile([128, 512], F32, tag="y", name="ysb")
                    nc.vector.tensor_add(
                        ysb[:], ps[:], bias_sb[:, co * 512:co * 512 + 512])
                    nc.sync.dma_start(
                        y_d[tb * 128:(tb + 1) * 128,
                            co * 512:(co + 1) * 512], ysb[:])

            y2st = cst.tile([128, 4 * C], F32, name="y2st")

            def pproj(k):
                # tokens 3072+512k .. +512 of staged -> y2 rows 512k..
                for tb in range(4):
                    g0 = 3072 + 512 * k + 128 * tb
                    for co in range(2):
                        ps = psQ.tile([128, 512], F32, tag="q", name="psp")
                        nc.tensor.matmul(
                            ps[:], staged[:, g0: g0 + 128],
                            wpo_sb[:, co * 512: co * 512 + 512],
                            start=True, stop=True)
                        nc.vector.tensor_copy(
                            y2st[:, tb * C + co * 512:
                                 tb * C + co * 512 + 512], ps[:])
                nc.sync.dma_start(
                    y2_d[512 * k: 512 * k + 512, :]
                    .rearrange("(tb p) n -> p tb n", p=128),
                    y2st[:].rearrange("p (tb n) -> p tb n", tb=4))

            qkv_block(0, 0, 1)
            p = attn_block(0, 0)
            for I in range(1, 4):       # qkv one step ahead of attn
                qkv_block(0, I, I + 1)
                np_, p = attn_block(0, I), None if p is None else p()
                p = np_
            qkv_block(1, 0, 1)
            np_, _ = attn_block(1, 0), p()
            p = np_
            exchange(0, 0, HTS, a2a_in[0], a2a_out[0])       # b0, hidden
            qkv_block(1, 1, 2)
            np_, _ = attn_block(1, 1), p()
            p = np_
            qkv_block(1, 2, 3)
            p()                         # staged b1 t<1024 complete
            exchange(1, T, 128, a2a_in[1], a2a_out[1])       # b1 1st half
            load_po(po0, a2a_out[0], HTS)
            qkv_block(1, 3, 4)
            p = attn_block(1, 2)
            proj_tb(0, po0, HTS)
            proj_tb(1, po0, HTS)
            p()
            pproj(0)               # tokens 1024-1535, local partial
            load_po(po1a, a2a_out[1], 128)
            p = attn_block(1, 3)
            p()
            pproj(1)               # tokens 1536-2047, local partial
            proj_tb(2, po1a, 128)


    nc.compile()
    return nc


def _prep_inputs(x, w_qkv, w_proj, b_proj):
    bf = ml_dtypes.bfloat16
    xT = np.ascontiguousarray(x.reshape(G, C).T).astype(bf)
    wpT = np.ascontiguousarray(w_proj.T).astype(bf)
    bias = np.ascontiguousarray(
        np.broadcast_to(b_proj.astype(np.float32), (128, C)))
    mask = (np.arange(896)[None, :] - 384 >=
            np.arange(128)[:, None]).astype(bf)
    scale = np.float32(HS ** -0.5)
    in_maps = []
    for c in range(NCORES):
        h0, h1 = 2 * c, 2 * c + 1
        cols = []
        for part, sc in ((slice(0, 64), scale), (slice(64, 128), None),
                         (slice(128, 192), None)):
            for h in (h0, h1):
                w = w_qkv[h, part, :]
                if sc is not None:
                    w = w * sc
                cols.append(np.ascontiguousarray(w.T))
        wc = np.concatenate(cols, axis=1).astype(bf)   # [C, 384]
        wpo = np.ascontiguousarray(wpT[128 * c:128 * (c + 1), :]).astype(bf)
        in_maps.append({"xT": xT, "wqkv": wc, "wpT": wpT,
                        "bias": bias, "mask": mask, "wpo": wpo})
    return in_maps


def _get_nc():
    global _CACHED
    if _CACHED is None:
        _CACHED = _build()
    return _CACHED


def run_on_cores(in_maps, **kwargs):
    nc = _get_nc()
    return bass_utils.run_bass_kernel_spmd(
        nc, in_maps, core_ids=list(range(NCORES)), **kwargs)


def kernel(x, w_qkv, w_proj, b_proj):
    in_maps = _prep_inputs(x, w_qkv, w_proj, b_proj)
    res = run_on_cores(in_maps)
    y = np.empty((B, T, C), dtype=np.float32)
    acc = None
    for c in range(NCORES):
        yc = res.results[c]["y"]
        y[0, HTS * c: HTS * (c + 1), :] = yc[0:256]
        y[1, 128 * c: 128 * (c + 1), :] = yc[256:384]
        y2 = res.results[c]["y2"]
        acc = y2 if acc is None else acc + y2
    y[1, 1024:2048, :] = acc + b_proj.astype(np.float32)[None, :]
    return y
